# revision 5
# baseline (speedup 1.0000x reference)
"""Causal self-attention (B=2, S=2048, D=1024, H=16) on 8 TRN2 NeuronCores.

Sharding: batch (2) x head-group (4 heads each) -> 8 cores. Each core computes
Q/K/V projections for its 4 heads, causal flash-attention, and a partial
output projection (its 256 columns of the concatenated head outputs against
the matching rows of Wo^T). Host sums the 4 partials per batch and adds the
bias terms (bv @ Wo.T + bo), which are x-independent.

All large inputs are packed host-side into ONE [128, 24576] fp16 tensor
(fp16 I/O halves HBM traffic vs f32; rel-err stays ~4e-4, far under the
2e-2 gate; fp8 was measured at 2.6e-2+ and rejected). Column map per
partition p:
  [     0:16384)  xT   s-major tiles: xt[p, sc, c, s] = x[b].T[128c+p, 512sc+s]
  [16384:18432)  wqT  tiles: wq[p, c, d] = Wq.T[:, sl][128c+p, d] (8 x 256)
  [18432:20480)  wkT  same for Wk
  [20480:22528)  wvT  same for Wv
  [22528:24576)  woT  tiles: wo[p, t, e] = Wo.T[sl, :][128t+p, e] (2 x 1024)
Biases travel in a tiny [128, 4] f32 side tensor (bq | bk halves).

DMA (software-pipelined): one full load before the hardware rep loop, then
inside each rep the NEXT iteration's inputs are reloaded region-by-region
from the otherwise-idle nc.sync (SP) HWDGE queue, each reload placed right
after its region's last reader (x[j] after attn(j); Wq/Wk+bias after
attn(2); Wv and Wo after attn(3), with Wo's reload BEFORE the final
out-projection, which then reads the freshly-loaded identical bytes). No
DMA may trail the body: the For_i back-edge tracks big_sb coarsely and
waits the last DMA's completion, so a trailing write stalls the next rep's
first matmuls by ~8us. Every rep still moves the full 10.5MB in / 4MB out.

Compute per core (all matmuls fp16 at 1 col/cycle, fp32 PSUM):
  - projections fp16; QT/KT evacuated to fp16 with bias fused (DVE)
  - scores^T[k,q] tiles via fp16 matmuls, 2 heads row-packed per 128
    partitions: lhsT base partitions 0/64 auto-derive tile_position
    (0,0)/(64,0), so the pair runs CONCURRENTLY in the PE's 2x row-tiling
    mode (measured 3ns apart on HW)
  - both heads' scores land in ONE [128,2,512] 2-bank PSUM tile -> ONE exp
    (ACT, 1/sqrt(dk) scale fused) and ONE mask multiply per k-tile; ACT
    per-instruction overhead (~260ns: PSUM access + decode) is paid once,
    and the score pair no longer serializes on PSUM-pool pressure
  - causal masking: multiplicative 0/1 fp16 mask, both heads per op
  - PV matmul fp16 with a ones column appended to V so the softmax
    denominator falls out of the same matmul (psum row 64)
  - attention emission is batched in 2-tile groups [sps x4][pv x4 +
    fillers]: entering/leaving the PE's 64-row tiling mode costs
    ~100-265ns, so scores (64-row) and PV/projections (128-row) run in
    stretches (the Tile scheduler reorders some of this; batching still
    measured ~3us)
  - per-head normalize chains (copy denom row -> reciprocal_approx_fast ->
    gpsimd partition_broadcast -> multiply), head-ordered so the first
    o_ps PSUM buffer releases after one chain latency
  - out-projection fp16 against Wo^T rows; fp16 output DMA per 512-row chunk

Projection and out-projection matmuls are emitted as generators of small
quanta that the attention loop drains at batch boundaries. V(j)'s
projection is drained INSIDE attn(j) (flushed before its straddle tiles,
its first readers — PE is in-order, so a later-queued producer would
deadlock). Per-s-chunk SBUF tiles keep cross-phase dependencies precise.
PSUM budget: pv(2) + scores(2x 2-bank) + o(2) = 8 banks.

Measured (HW trace, steady-state rep period): baseline 167us -> 143us
(merged exp ~10us, pipelined DMA ~11us incl. Wo placement, batching ~3us).
PE is the bottleneck (~95% busy; fp16 stream floor ~108us/rep).
"""

import numpy as np

N_CORES = 8
B, S, D = 2, 2048, 1024
H_PER_CORE = 4
DSL = 256
NC_TILES = 8
SCH = 512
NSCH = S // SCH
NST = S // 128

XT_O = 0
WQ_O = 16384
WK_O = WQ_O + 2048
WV_O = WK_O + 2048
WO_O = WV_O + 2048
IN_COLS = WO_O + 2048  # 24576
SB_COLS = IN_COLS + 2048  # second Wo slot (SBUF only; DRAM stays 24576)

_cache = {}


def _build(reps=1, dma="pipe", drain=(1, 1, 1, 1), pools=(2, 2, 2), ep_bufs=6):
    import contextlib
    import concourse.mybir as mybir
    import concourse.tile as tile
    from concourse import bacc

    f32 = mybir.dt.float32
    f32r = mybir.dt.float32r
    f16 = mybir.dt.float16
    EXP = mybir.ActivationFunctionType.Exp

    nc = bacc.Bacc("TRN2", target_bir_lowering=False, debug=False,
                   num_devices=N_CORES)

    big = nc.dram_tensor("big", [128, IN_COLS], f16, kind="ExternalInput").ap()
    bqk = nc.dram_tensor("bqk", [128, 4], f32, kind="ExternalInput").ap()
    y = nc.dram_tensor("y", [S, D], f16, kind="ExternalOutput").ap()

    with tile.TileContext(nc) as tc:
        with contextlib.ExitStack() as ctx:
            singles = ctx.enter_context(tc.tile_pool(name="singles", bufs=1))
            work = ctx.enter_context(tc.tile_pool(name="work", bufs=1))

            big_sb = singles.tile([128, SB_COLS], f16)
            # x packed s-major: [sc, c, 512] so one 1MB DMA delivers
            # everything proj(sc) needs (QK + V of chunk sc).
            xt_sb = big_sb[:, XT_O:WQ_O].rearrange(
                "p (sc c s) -> p sc c s", sc=NSCH, c=NC_TILES)
            wq_sb = big_sb[:, WQ_O:WK_O].rearrange("p (c d) -> p c d", c=NC_TILES)
            wk_sb = big_sb[:, WK_O:WV_O].rearrange("p (c d) -> p c d", c=NC_TILES)
            wv_sb = big_sb[:, WV_O:WO_O].rearrange("p (c d) -> p c d", c=NC_TILES)
            # Wo double-buffered: bodies alternate slots, each body
            # reloads the OTHER slot (read by the previous body) right
            # at body start, so outp(3) never waits on a DMA and no
            # reload sits near the body tail.
            wo_sb2 = [big_sb[:, WO_O:IN_COLS].rearrange("p (t e) -> p t e", t=2),
                      big_sb[:, IN_COLS:SB_COLS].rearrange("p (t e) -> p t e", t=2)]
            bqk_sb = singles.tile([128, 4], f32)

            # per-s-chunk tiles -> precise cross-phase dependencies
            qt_sb = [work.tile([128, 2, SCH], f16, name=f"qt{j}", tag=f"qt{j}")
                     for j in range(NSCH)]
            kt_sb = [work.tile([128, 2, SCH], f16, name=f"kt{j}", tag=f"kt{j}")
                     for j in range(NSCH)]
            v_sb = [work.tile([128, 4, 260], f16, name=f"v{j}", tag=f"v{j}")
                    for j in range(NSCH)]
            att_sb = [[work.tile([128, SCH], f16, name=f"att{j}_{p}", tag=f"att{j}_{p}")
                       for p in range(2)] for j in range(NSCH)]
            masks = [singles.tile([128, 2, SCH], f16, name=f"mask{m}", tag=f"mask{m}")
                     for m in range(4)]

            # causal 0/1 masks: block row k (partition), col q;
            # valid iff q - k - 128*m >= 0. Two identical halves so ONE
            # DVE multiply masks both heads of a merged exp tile.
            for m in range(4):
                nc.gpsimd.memset(masks[m], 1.0)
                for h in range(2):
                    nc.gpsimd.affine_select(
                        out=masks[m][:, h, :], in_=masks[m][:, h, :],
                        compare_op=mybir.AluOpType.is_ge, fill=0.0,
                        base=-128 * m, pattern=[[1, SCH]], channel_multiplier=-1)
            # ones columns of V (col 64 of each head slot), written once:
            # per-rep V copies only touch cols 0..63 of each slot.
            for j in range(NSCH):
                nc.gpsimd.memset(v_sb[j], 1.0)

            def dma_in():
                # All INPUT loads ride the scalar HWDGE queue, all y stores
                # ride the sync queue: HWDGE queues are FIFO per issuing
                # engine, so mixing directions would park rep i+1's first
                # input load behind rep i's last y store. x s-chunk 0 +
                # Wq/Wk/Wv first (first matmuls need them); Wo last and
                # separate: its last reader is outp(3) at the very end of a
                # rep, so a fused weight DMA would serialize rep i+1's whole
                # input load behind rep i's tail.
                nc.scalar.dma_start(out=big_sb[:, XT_O:XT_O + 4096],
                                    in_=big[:, XT_O:XT_O + 4096])
                nc.scalar.dma_start(out=big_sb[:, WQ_O:WO_O], in_=big[:, WQ_O:WO_O])
                nc.scalar.dma_start(out=bqk_sb, in_=bqk)
                for sc in range(1, NSCH):
                    cs = slice(XT_O + 4096 * sc, XT_O + 4096 * (sc + 1))
                    nc.scalar.dma_start(out=big_sb[:, cs], in_=big[:, cs])
                nc.scalar.dma_start(out=big_sb[:, WO_O:IN_COLS], in_=big[:, WO_O:IN_COLS])
                nc.scalar.dma_start(out=big_sb[:, IN_COLS:SB_COLS], in_=big[:, WO_O:IN_COLS])

            if dma in ("once", "pipe"):
                dma_in()

            def reload(c0_, c1_):
                # next-iteration input prefetch on the (otherwise idle) SP
                # HWDGE queue, placed right after the region's last reader
                # so the FIFO never head-of-line blocks.
                nc.sync.dma_start(out=big_sb[:, c0_:c1_], in_=big[:, c0_:c1_])

            def body(par=0):
                wo_sb = wo_sb2[par]
                with contextlib.ExitStack() as bctx:
                    if dma == "pipe":
                        # refresh the slot the PREVIOUS body read; its
                        # readers (prev outp) just finished, the DMA
                        # runs under this body's early compute.
                        dst = slice(IN_COLS, SB_COLS) if par == 0 else slice(WO_O, IN_COLS)
                        nc.sync.dma_start(out=big_sb[:, dst], in_=big[:, WO_O:IN_COLS])
                    if dma == "loop":
                        dma_in()

                    pv = bctx.enter_context(tc.tile_pool(name="pv", bufs=pools[0], space="PSUM"))
                    sp_ = bctx.enter_context(tc.tile_pool(name="sp", bufs=pools[1], space="PSUM"))
                    op_ = bctx.enter_context(tc.tile_pool(name="op", bufs=pools[2], space="PSUM"))
                    ep = bctx.enter_context(tc.tile_pool(name="ep", bufs=ep_bufs))
                    bp = bctx.enter_context(tc.tile_pool(name="bp", bufs=4))
                    yo = bctx.enter_context(tc.tile_pool(name="yo", bufs=2))

                    def qk_gen(sc, halves=(0, 1)):
                        """Q/K projection for s-chunk sc as small PE quanta.

                        halves selects head-pair halves: attn(sc) pair p only
                        reads half p, so half 1 can be deferred into attn(sc)
                        pair 0's drain slots.
                        """
                        for half in halves:
                            for w_sb, dst, boff in ((wq_sb, qt_sb[sc], 0),
                                                    (wk_sb, kt_sb[sc], 2)):
                                ps = pv.tile([128, SCH], f32, name="pj", tag="pv")
                                for c in range(NC_TILES):
                                    nc.tensor.matmul(
                                        ps, lhsT=w_sb[:, c, 128 * half:128 * (half + 1)],
                                        rhs=xt_sb[:, sc, c, :],
                                        start=(c == 0), stop=(c == NC_TILES - 1))
                                    if c % 2:
                                        yield
                                nc.vector.tensor_scalar_add(
                                    dst[:, half, :], ps,
                                    bqk_sb[:, boff + half:boff + half + 1])
                                yield

                    def v_gen(sc):
                        """V projection for s-chunk sc as small PE quanta."""
                        for t4 in range(4):
                            v_ps = pv.tile([128, DSL], f32, name="vps", tag="pv")
                            for c in range(NC_TILES):
                                nc.tensor.matmul(
                                    v_ps, lhsT=xt_sb[:, sc, c, 128 * t4:128 * (t4 + 1)],
                                    rhs=wv_sb[:, c, :], start=(c == 0),
                                    stop=(c == NC_TILES - 1))
                                if c % 2:
                                    yield
                            nc.any.tensor_copy(
                                out=v_sb[sc].rearrange("p t (h e) -> p t h e", h=4)[:, t4, :, 0:64],
                                in_=v_ps.rearrange("p (h e) -> p h e", h=4))
                            yield

                    def outp_gen(j):
                        """Out-projection for q-chunk j as small PE quanta."""
                        y_sb = yo.tile([128, 4, D], f16, name="ysb", tag="ysb")
                        for t4 in range(4):
                            for e in range(2):
                                es = slice(512 * e, 512 * (e + 1))
                                y_ps = pv.tile([128, 512], f32, name="yps", tag="pv")
                                for pair in range(2):
                                    nc.tensor.matmul(
                                        y_ps, lhsT=att_sb[j][pair][:, 128 * t4:128 * (t4 + 1)],
                                        rhs=wo_sb[:, pair, es],
                                        start=(pair == 0), stop=(pair == 1))
                                if j == NSCH - 1:
                                    # rep tail: ACT is exp-idle here and DVE
                                    # is busy with the normalize chains
                                    nc.scalar.copy(out=y_sb[:, t4, es], in_=y_ps)
                                else:
                                    nc.any.tensor_copy(out=y_sb[:, t4, es], in_=y_ps)
                                yield
                        nc.sync.dma_start(
                            out=y[SCH * j:SCH * (j + 1), :].rearrange("(t p) e -> p t e", p=128),
                            in_=y_sb)
                        yield

                    def attn(j, bg, bg_early=None, early_rate=2, bg_p1=None):
                        # bg_early: quanta that must finish before the PV of
                        # tile 4j (V(j) work: this chunk's straddle tiles are
                        # its first readers). Paced per 2-tile batch, force-
                        # flushed before the first straddle PV. bg_p1:
                        # quanta only pair 1 depends on (its Q/K half) —
                        # drained during pair 0, flushed at the pair boundary.
                        #
                        # Emission is batched in 2-tile groups: [sps sps sps
                        # sps][fillers + pv pv pv pv] so the PE's 64-row
                        # tiling mode (scores) and 128-row mode (everything
                        # else) each run in stretches — the mode-switch
                        # bubble is paid once per batch, not once per tile.
                        T = 4 * (j + 1)
                        nd = drain[j]
                        early_left = bg_early
                        for pair in range(2):
                            if pair == 1 and bg_p1 is not None:
                                for _ in bg_p1:
                                    pass
                                bg_p1 = None
                            o_ps = [op_.tile([65, SCH], f32, name=f"ops{h}", tag="o")
                                    for h in range(2)]
                            pend = []

                            def emit_pv(exps, t, c0):
                                cs_ = slice(c0, SCH)
                                for h in range(2):
                                    hl = 2 * pair + h
                                    nc.tensor.matmul(
                                        o_ps[h][:, cs_], lhsT=v_sb[t // 4][:, t % 4, 65 * hl:65 * hl + 65],
                                        rhs=exps[:, h, cs_], start=(t == 0), stop=(t == T - 1))

                            def flush_early_for(t_):
                                # PV of straddle tile t_ >= 4j reads v_sb[j]:
                                # all of V(j)'s quanta must be issued first
                                # (PE is in-order; a later-queued producer
                                # would deadlock the consumer).
                                nonlocal early_left
                                if early_left is not None and t_ - 4 * j >= 0:
                                    for _ in early_left:
                                        pass
                                    early_left = None

                            for t in range(T):
                                m = t - 4 * j
                                # straddle tile m: columns < 128m are fully
                                # masked -> skip them in scores/exp/mask/PV
                                c0 = 128 * m if m > 0 else 0
                                cs_ = slice(c0, SCH)
                                # both heads in ONE 2-bank PSUM tile -> one
                                # exp + one mask per tile (ACT per-inst
                                # overhead halved; scores pair stays
                                # row-tile-concurrent in the PE array)
                                s_ps = sp_.tile([128, 2, SCH], f32, name="sps", tag="s")
                                for h in range(2):
                                    hp = slice(64 * h, 64 * (h + 1))
                                    nc.tensor.matmul(
                                        s_ps[:, h, cs_],
                                        lhsT=kt_sb[t // 4][hp, pair, 128 * (t % 4):128 * (t % 4 + 1)],
                                        rhs=qt_sb[j][hp, pair, cs_], start=True, stop=True)
                                exps = ep.tile([128, 2, SCH], f16, name="exps", tag="e")
                                nc.scalar.activation(out=exps[:, :, cs_], in_=s_ps[:, :, cs_],
                                                     func=EXP, scale=0.125)
                                if m >= 0:
                                    nc.vector.tensor_mul(exps[:, :, cs_], exps[:, :, cs_],
                                                         masks[m][:, :, cs_])
                                pend.append((exps, t, c0))
                                if t % 2 == 1:
                                    # PVs first (one sps->ops switch), then
                                    # fillers (128-row like ops: free).
                                    while len(pend) > 2:
                                        e_, t_, c_ = pend.pop(0)
                                        flush_early_for(t_)
                                        emit_pv(e_, t_, c_)
                                    if early_left is not None:
                                        for _ in range(2 * early_rate):
                                            next(early_left, None)
                                    if bg_p1 is not None:
                                        for _ in range(2):
                                            next(bg_p1, None)
                                    for _ in range(2 * nd):
                                        next(bg, None)
                            for e_, t_, c_ in pend:
                                flush_early_for(t_)
                                emit_pv(e_, t_, c_)

                            # normalize: att = O[0:64] * bcast(1/denom).
                            # Head-ordered chains so head 0's o_ps releases
                            # (and the next pair's PV unblocks, with op
                            # bufs=3) after ONE chain latency, not two.
                            for h in range(2):
                                bc = bp.tile([128, SCH], f32, name=f"bc{h}", tag="bc")
                                nc.vector.tensor_copy(out=bc[0:1, :], in_=o_ps[h][64:65, :])
                                nc.vector.reciprocal_approx_fast(
                                    out=bc[0:1, :], in_=bc[0:1, :])
                                nc.gpsimd.partition_broadcast(
                                    out_ap=bc[0:64, :], in_ap=bc[0:1, :])
                                nc.vector.tensor_mul(
                                    att_sb[j][pair][64 * h:64 * (h + 1), :],
                                    o_ps[h][0:64, :], bc[0:64, :])

                    def drain_all(bg):
                        for _ in bg:
                            pass

                    def chain(*gens):
                        for g in gens:
                            yield from g

                    # Only pair 0's Q/K half runs before attn(0); pair 1's
                    # half and V(0) drain inside attn(0) itself (V paced one
                    # group ahead of its PV). Later chunks drain V(j) early
                    # (their straddle tiles read it) plus outp(j-1) and
                    # QK(j+1) quanta between k-tiles — keeps PE fed in the
                    # late, filler-starved chunks and shrinks the serial
                    # prologue at each rep boundary.
                    drain_all(qk_gen(0, halves=(0,)))
                    for j in range(NSCH):
                        gens = []
                        if j > 0:
                            gens.append(outp_gen(j - 1))
                        if j + 1 < NSCH:
                            gens.append(qk_gen(j + 1))
                        bg = chain(*gens)
                        attn(j, bg, bg_early=v_gen(j),
                             early_rate=5 if j == 0 else 2,
                             bg_p1=qk_gen(0, halves=(1,)) if j == 0 else None)
                        drain_all(bg)
                        if dma == "pipe":
                            # reload regions whose last reader just drained:
                            # x[j] (v_gen(j)); after attn(2) also Wq/Wk+bias
                            # (qk_gen(3) drained inside attn(2)); after
                            # attn(3) also Wv (v_gen(3)) and Wo. Wo's reload
                            # sits BEFORE outp(3), which then reads the
                            # freshly-loaded (identical) bytes — no DMA may
                            # trail the body: the loop back-edge tracks
                            # big_sb coarsely, so a post-body write would
                            # stall the next rep's first matmuls (~8us).
                            reload(XT_O + 4096 * j, XT_O + 4096 * (j + 1))
                            if j == 2:
                                reload(WQ_O, WV_O)
                                nc.sync.dma_start(out=bqk_sb, in_=bqk)
                            if j == 3:
                                reload(WV_O, WO_O)
                    drain_all(outp_gen(NSCH - 1))

            if reps == 1:
                body(0)
            elif reps % 2 == 0:
                # body unrolled x2 per hardware-loop iteration: the For_i
                # back-edge costs ~7us (per-engine drains/branch + waiting
                # the last DMA's completion for sem-lane recycling) — paid
                # once per TWO reps this way. The body-to-body seam inside
                # an iteration is a plain instruction-stream transition the
                # Tile scheduler can overlap across. Measured 143.4 ->
                # 141.5us/rep.
                with tc.For_i(0, reps // 2, 1):
                    body(0)
                    body(1)
            else:
                with tc.For_i(0, reps, 1):
                    body()

    nc.compile()
    return nc


def _get_nc(reps=1, **kw):
    key = (reps, tuple(sorted(kw.items())))
    if key not in _cache:
        _cache[key] = _build(reps, **kw)
    return _cache[key]


def _tiles(a, nt):
    # [nt*128, w] -> [128, nt*w] with [p, t*w:t*w+w] = a[128t+p, :]
    w = a.shape[1]
    return a.reshape(nt, 128, w).transpose(1, 0, 2).reshape(128, nt * w)


def make_in_maps(x, Wq, bq, Wk, bk, Wv, bv, Wo, bo):
    """Shard full inputs into 8 per-core input dicts (fp16 payload)."""
    in_maps = []
    for core in range(N_CORES):
        b, g = core // 4, core % 4
        sl = slice(DSL * g, DSL * (g + 1))
        # x s-major: cols [sc, c, s] with xt[p, sc, c, s] = x[b].T[128c+p, 512sc+s]
        xsm = x[b].T.reshape(8, 128, 4, 512).transpose(1, 2, 0, 3).reshape(128, 16384)
        big = np.concatenate([
            xsm,
            _tiles(np.ascontiguousarray(Wq[sl, :].T), 8),
            _tiles(np.ascontiguousarray(Wk[sl, :].T), 8),
            _tiles(np.ascontiguousarray(Wv[sl, :].T), 8),
            _tiles(np.ascontiguousarray(Wo[:, sl].T), 2),
        ], axis=1).astype(np.float16)
        bqk = np.concatenate([bq[sl].reshape(2, 128).T, bk[sl].reshape(2, 128).T],
                             axis=1)
        in_maps.append({"big": big, "bqk": np.ascontiguousarray(bqk)})
    return in_maps


def kernel(x, Wq, bq, Wk, bk, Wv, bv, Wo, bo):
    from concourse.bass_utils import run_bass_kernel_spmd

    x = np.asarray(x, dtype=np.float32)
    Wq, bq = np.asarray(Wq, np.float32), np.asarray(bq, np.float32)
    Wk, bk = np.asarray(Wk, np.float32), np.asarray(bk, np.float32)
    Wv, bv = np.asarray(Wv, np.float32), np.asarray(bv, np.float32)
    Wo, bo = np.asarray(Wo, np.float32), np.asarray(bo, np.float32)

    nc = _get_nc()
    in_maps = make_in_maps(x, Wq, bq, Wk, bk, Wv, bv, Wo, bo)
    res = run_bass_kernel_spmd(nc, in_maps, core_ids=list(range(N_CORES)))

    cvec = bv @ Wo.T + bo  # x-independent bias contribution
    out = np.zeros((B, S, D), dtype=np.float32)
    for core in range(N_CORES):
        out[core // 4] += res.results[core]["y"].astype(np.float32)
    out += cvec[None, None, :]
    return out



# revision 6
# speedup vs baseline: 2.2703x; 2.2703x over previous
"""Causal self-attention (B=2, S=2048, D=1024, H=16) on 8 TRN2 NeuronCores.

Sharding: batch (2) x head-group (4 heads each) -> 8 cores. Each core computes
Q/K/V projections for its 4 heads, causal flash-attention, and a partial
output projection (its 256 columns of the concatenated head outputs against
the matching rows of Wo^T). Host sums the 4 partials per batch and adds the
bias terms (bv @ Wo.T + bo), which are x-independent.

All large inputs are packed host-side into ONE [128, 24576] fp16 tensor
(fp16 I/O halves HBM traffic vs f32; rel-err stays ~4e-4, far under the
2e-2 gate; fp8 was measured at 2.6e-2+ and rejected). Column map per
partition p:
  [     0:16384)  xT   s-major tiles: xt[p, sc, c, s] = x[b].T[128c+p, 512sc+s]
  [16384:18432)  wqT  tiles: wq[p, c, d] = Wq.T[:, sl][128c+p, d] (8 x 256)
  [18432:20480)  wkT  same for Wk
  [20480:22528)  wvT  same for Wv
  [22528:24576)  woT  tiles: wo[p, t, e] = Wo.T[sl, :][128t+p, e] (2 x 1024)
Biases travel in a tiny [128, 4] f32 side tensor (bq | bk halves).

DMA (software-pipelined): one full load before the hardware rep loop, then
inside each rep the NEXT iteration's inputs are reloaded region-by-region
from the otherwise-idle nc.sync (SP) HWDGE queue, each reload placed right
after its region's last reader (x[j] after attn(j); Wq/Wk+bias after
attn(2); Wv and Wo after attn(3), with Wo's reload BEFORE the final
out-projection, which then reads the freshly-loaded identical bytes). No
DMA may trail the body: the For_i back-edge tracks big_sb coarsely and
waits the last DMA's completion, so a trailing write stalls the next rep's
first matmuls by ~8us. Every rep still moves the full 10.5MB in / 4MB out.

Compute per core (all matmuls fp16 at 1 col/cycle, fp32 PSUM):
  - projections fp16; QT/KT evacuated to fp16 with bias fused (DVE)
  - scores^T[k,q] tiles via fp16 matmuls, 2 heads row-packed per 128
    partitions: lhsT base partitions 0/64 auto-derive tile_position
    (0,0)/(64,0), so the pair runs CONCURRENTLY in the PE's 2x row-tiling
    mode (measured 3ns apart on HW)
  - both heads' scores land in ONE [128,2,512] 2-bank PSUM tile -> ONE exp
    (ACT, 1/sqrt(dk) scale fused) and ONE mask multiply per k-tile; ACT
    per-instruction overhead (~260ns: PSUM access + decode) is paid once,
    and the score pair no longer serializes on PSUM-pool pressure
  - causal masking: multiplicative 0/1 fp16 mask, both heads per op
  - PV matmul fp16 with a ones column appended to V so the softmax
    denominator falls out of the same matmul (psum row 64)
  - attention emission is batched in 2-tile groups [sps x4][pv x4 +
    fillers]: entering/leaving the PE's 64-row tiling mode costs
    ~100-265ns, so scores (64-row) and PV/projections (128-row) run in
    stretches (the Tile scheduler reorders some of this; batching still
    measured ~3us)
  - per-head normalize chains (copy denom row -> reciprocal_approx_fast ->
    gpsimd partition_broadcast -> multiply), head-ordered so the first
    o_ps PSUM buffer releases after one chain latency
  - out-projection fp16 against Wo^T rows; fp16 output DMA per 512-row chunk

Projection and out-projection matmuls are emitted as generators of small
quanta that the attention loop drains at batch boundaries. V(j)'s
projection is drained INSIDE attn(j) (flushed before its straddle tiles,
its first readers — PE is in-order, so a later-queued producer would
deadlock). Per-s-chunk SBUF tiles keep cross-phase dependencies precise.
PSUM budget: pv(2) + scores(2x 2-bank) + o(2) = 8 banks.

Measured (HW trace, steady-state rep period): baseline 167us -> 143us
(merged exp ~10us, pipelined DMA ~11us incl. Wo placement, batching ~3us).
PE is the bottleneck (~95% busy; fp16 stream floor ~108us/rep).
"""

import numpy as np

N_CORES = 8
B, S, D = 2, 2048, 1024
H_PER_CORE = 4
DSL = 256
NC_TILES = 8
SCH = 512
NSCH = S // SCH
NST = S // 128

XT_O = 0
WQ_O = 16384
WK_O = WQ_O + 2048
WV_O = WK_O + 2048
WO_O = WV_O + 2048
IN_COLS = WO_O + 2048  # 24576
SB_COLS = IN_COLS + 2048  # second Wo slot (SBUF only; DRAM stays 24576)

_cache = {}


def _build(reps=1, dma="pipe", drain=(1, 1, 1, 1), pools=(2, 2, 2), ep_bufs=6):
    import contextlib
    import concourse.mybir as mybir
    import concourse.tile as tile
    from concourse import bacc

    f32 = mybir.dt.float32
    f32r = mybir.dt.float32r
    f16 = mybir.dt.float16
    EXP = mybir.ActivationFunctionType.Exp

    nc = bacc.Bacc("TRN2", target_bir_lowering=False, debug=False,
                   num_devices=N_CORES)

    big = nc.dram_tensor("big", [128, IN_COLS], f16, kind="ExternalInput").ap()
    bqk = nc.dram_tensor("bqk", [128, 4], f32, kind="ExternalInput").ap()
    y = nc.dram_tensor("y", [S, D], f16, kind="ExternalOutput").ap()

    with tile.TileContext(nc) as tc:
        with contextlib.ExitStack() as ctx:
            singles = ctx.enter_context(tc.tile_pool(name="singles", bufs=1))
            work = ctx.enter_context(tc.tile_pool(name="work", bufs=1))

            big_sb = singles.tile([128, SB_COLS], f16)
            # x packed s-major: [sc, c, 512] so one 1MB DMA delivers
            # everything proj(sc) needs (QK + V of chunk sc).
            xt_sb = big_sb[:, XT_O:WQ_O].rearrange(
                "p (sc c s) -> p sc c s", sc=NSCH, c=NC_TILES)
            wq_sb = big_sb[:, WQ_O:WK_O].rearrange("p (c d) -> p c d", c=NC_TILES)
            wk_sb = big_sb[:, WK_O:WV_O].rearrange("p (c d) -> p c d", c=NC_TILES)
            wv_sb = big_sb[:, WV_O:WO_O].rearrange("p (c d) -> p c d", c=NC_TILES)
            # Wo double-buffered: bodies alternate slots, each body
            # reloads the OTHER slot (read by the previous body) right
            # at body start, so outp(3) never waits on a DMA and no
            # reload sits near the body tail.
            wo_sb2 = [big_sb[:, WO_O:IN_COLS].rearrange("p (t e) -> p t e", t=2),
                      big_sb[:, IN_COLS:SB_COLS].rearrange("p (t e) -> p t e", t=2)]
            bqk_sb = singles.tile([128, 4], f32)

            # per-s-chunk tiles -> precise cross-phase dependencies
            qt_sb = [work.tile([128, 2, SCH], f16, name=f"qt{j}", tag=f"qt{j}")
                     for j in range(NSCH)]
            kt_sb = [work.tile([128, 2, SCH], f16, name=f"kt{j}", tag=f"kt{j}")
                     for j in range(NSCH)]
            v_sb = [work.tile([128, 4, 260], f16, name=f"v{j}", tag=f"v{j}")
                    for j in range(NSCH)]
            att_sb = [[work.tile([128, SCH], f16, name=f"att{j}_{p}", tag=f"att{j}_{p}")
                       for p in range(2)] for j in range(NSCH)]
            masks = [singles.tile([128, 2, SCH], f16, name=f"mask{m}", tag=f"mask{m}")
                     for m in range(4)]

            # causal 0/1 masks: block row k (partition), col q;
            # valid iff q - k - 128*m >= 0. Two identical halves so ONE
            # DVE multiply masks both heads of a merged exp tile.
            for m in range(4):
                nc.gpsimd.memset(masks[m], 1.0)
                for h in range(2):
                    nc.gpsimd.affine_select(
                        out=masks[m][:, h, :], in_=masks[m][:, h, :],
                        compare_op=mybir.AluOpType.is_ge, fill=0.0,
                        base=-128 * m, pattern=[[1, SCH]], channel_multiplier=-1)
            # ones columns of V (col 64 of each head slot), written once:
            # per-rep V copies only touch cols 0..63 of each slot.
            for j in range(NSCH):
                nc.gpsimd.memset(v_sb[j], 1.0)

            def dma_in():
                # All INPUT loads ride the scalar HWDGE queue, all y stores
                # ride the sync queue: HWDGE queues are FIFO per issuing
                # engine, so mixing directions would park rep i+1's first
                # input load behind rep i's last y store. x s-chunk 0 +
                # Wq/Wk/Wv first (first matmuls need them); Wo last and
                # separate: its last reader is outp(3) at the very end of a
                # rep, so a fused weight DMA would serialize rep i+1's whole
                # input load behind rep i's tail.
                nc.scalar.dma_start(out=big_sb[:, XT_O:XT_O + 4096],
                                    in_=big[:, XT_O:XT_O + 4096])
                nc.scalar.dma_start(out=big_sb[:, WQ_O:WO_O], in_=big[:, WQ_O:WO_O])
                nc.scalar.dma_start(out=bqk_sb, in_=bqk)
                for sc in range(1, NSCH):
                    cs = slice(XT_O + 4096 * sc, XT_O + 4096 * (sc + 1))
                    nc.scalar.dma_start(out=big_sb[:, cs], in_=big[:, cs])
                nc.scalar.dma_start(out=big_sb[:, WO_O:IN_COLS], in_=big[:, WO_O:IN_COLS])
                nc.scalar.dma_start(out=big_sb[:, IN_COLS:SB_COLS], in_=big[:, WO_O:IN_COLS])

            if dma in ("once", "pipe"):
                dma_in()

            def reload(c0_, c1_):
                # next-iteration input prefetch on the (otherwise idle) SP
                # HWDGE queue, placed right after the region's last reader
                # so the FIFO never head-of-line blocks.
                nc.sync.dma_start(out=big_sb[:, c0_:c1_], in_=big[:, c0_:c1_])

            def body(par=0):
                wo_sb = wo_sb2[par]
                with contextlib.ExitStack() as bctx:
                    if dma == "pipe":
                        # refresh the slot the PREVIOUS body read; its
                        # readers (prev outp) just finished, the DMA
                        # runs under this body's early compute.
                        dst = slice(IN_COLS, SB_COLS) if par == 0 else slice(WO_O, IN_COLS)
                        nc.sync.dma_start(out=big_sb[:, dst], in_=big[:, WO_O:IN_COLS])
                    if dma == "loop":
                        dma_in()

                    pv = bctx.enter_context(tc.tile_pool(name="pv", bufs=pools[0], space="PSUM"))
                    sp_ = bctx.enter_context(tc.tile_pool(name="sp", bufs=pools[1], space="PSUM"))
                    op_ = bctx.enter_context(tc.tile_pool(name="op", bufs=pools[2], space="PSUM"))
                    ep = bctx.enter_context(tc.tile_pool(name="ep", bufs=ep_bufs))
                    bp = bctx.enter_context(tc.tile_pool(name="bp", bufs=4))
                    yo = bctx.enter_context(tc.tile_pool(name="yo", bufs=2))

                    def qk_gen(sc, halves=(0, 1)):
                        """Q/K projection for s-chunk sc as small PE quanta.

                        halves selects head-pair halves: attn(sc) pair p only
                        reads half p, so half 1 can be deferred into attn(sc)
                        pair 0's drain slots.
                        """
                        for half in halves:
                            for w_sb, dst, boff in ((wq_sb, qt_sb[sc], 0),
                                                    (wk_sb, kt_sb[sc], 2)):
                                ps = pv.tile([128, SCH], f32, name="pj", tag="pv")
                                for c in range(NC_TILES):
                                    nc.tensor.matmul(
                                        ps, lhsT=w_sb[:, c, 128 * half:128 * (half + 1)],
                                        rhs=xt_sb[:, sc, c, :],
                                        start=(c == 0), stop=(c == NC_TILES - 1))
                                    if c % 2:
                                        yield
                                nc.vector.tensor_scalar_add(
                                    dst[:, half, :], ps,
                                    bqk_sb[:, boff + half:boff + half + 1])
                                yield

                    def v_gen(sc):
                        """V projection for s-chunk sc as small PE quanta."""
                        for t4 in range(4):
                            v_ps = pv.tile([128, DSL], f32, name="vps", tag="pv")
                            for c in range(NC_TILES):
                                nc.tensor.matmul(
                                    v_ps, lhsT=xt_sb[:, sc, c, 128 * t4:128 * (t4 + 1)],
                                    rhs=wv_sb[:, c, :], start=(c == 0),
                                    stop=(c == NC_TILES - 1))
                                if c % 2:
                                    yield
                            nc.any.tensor_copy(
                                out=v_sb[sc].rearrange("p t (h e) -> p t h e", h=4)[:, t4, :, 0:64],
                                in_=v_ps.rearrange("p (h e) -> p h e", h=4))
                            yield

                    def outp_gen(j):
                        """Out-projection for q-chunk j as small PE quanta."""
                        y_sb = yo.tile([128, 4, D], f16, name="ysb", tag="ysb")
                        for t4 in range(4):
                            for e in range(2):
                                es = slice(512 * e, 512 * (e + 1))
                                y_ps = pv.tile([128, 512], f32, name="yps", tag="pv")
                                for pair in range(2):
                                    nc.tensor.matmul(
                                        y_ps, lhsT=att_sb[j][pair][:, 128 * t4:128 * (t4 + 1)],
                                        rhs=wo_sb[:, pair, es],
                                        start=(pair == 0), stop=(pair == 1))
                                if j == NSCH - 1:
                                    # rep tail: ACT is exp-idle here and DVE
                                    # is busy with the normalize chains
                                    nc.scalar.copy(out=y_sb[:, t4, es], in_=y_ps)
                                else:
                                    nc.any.tensor_copy(out=y_sb[:, t4, es], in_=y_ps)
                                yield
                        nc.sync.dma_start(
                            out=y[SCH * j:SCH * (j + 1), :].rearrange("(t p) e -> p t e", p=128),
                            in_=y_sb)
                        yield

                    def attn(j, bg, bg_early=None, early_rate=2, bg_p1=None):
                        # bg_early: quanta that must finish before the PV of
                        # tile 4j (V(j) work: this chunk's straddle tiles are
                        # its first readers). Paced per 2-tile batch, force-
                        # flushed before the first straddle PV. bg_p1:
                        # quanta only pair 1 depends on (its Q/K half) —
                        # drained during pair 0, flushed at the pair boundary.
                        #
                        # Emission is batched in 2-tile groups: [sps sps sps
                        # sps][fillers + pv pv pv pv] so the PE's 64-row
                        # tiling mode (scores) and 128-row mode (everything
                        # else) each run in stretches — the mode-switch
                        # bubble is paid once per batch, not once per tile.
                        T = 4 * (j + 1)
                        nd = drain[j]
                        early_left = bg_early
                        for pair in range(2):
                            if pair == 1 and bg_p1 is not None:
                                for _ in bg_p1:
                                    pass
                                bg_p1 = None
                            o_ps = [op_.tile([65, SCH], f32, name=f"ops{h}", tag="o")
                                    for h in range(2)]
                            pend = []

                            def emit_pv(exps, t, c0):
                                cs_ = slice(c0, SCH)
                                for h in range(2):
                                    hl = 2 * pair + h
                                    nc.tensor.matmul(
                                        o_ps[h][:, cs_], lhsT=v_sb[t // 4][:, t % 4, 65 * hl:65 * hl + 65],
                                        rhs=exps[:, h, cs_], start=(t == 0), stop=(t == T - 1))

                            def flush_early_for(t_):
                                # PV of straddle tile t_ >= 4j reads v_sb[j]:
                                # all of V(j)'s quanta must be issued first
                                # (PE is in-order; a later-queued producer
                                # would deadlock the consumer).
                                nonlocal early_left
                                if early_left is not None and t_ - 4 * j >= 0:
                                    for _ in early_left:
                                        pass
                                    early_left = None

                            for t in range(T):
                                m = t - 4 * j
                                # straddle tile m: columns < 128m are fully
                                # masked -> skip them in scores/exp/mask/PV
                                c0 = 128 * m if m > 0 else 0
                                cs_ = slice(c0, SCH)
                                # both heads in ONE 2-bank PSUM tile -> one
                                # exp + one mask per tile (ACT per-inst
                                # overhead halved; scores pair stays
                                # row-tile-concurrent in the PE array)
                                s_ps = sp_.tile([128, 2, SCH], f32, name="sps", tag="s")
                                for h in range(2):
                                    hp = slice(64 * h, 64 * (h + 1))
                                    nc.tensor.matmul(
                                        s_ps[:, h, cs_],
                                        lhsT=kt_sb[t // 4][hp, pair, 128 * (t % 4):128 * (t % 4 + 1)],
                                        rhs=qt_sb[j][hp, pair, cs_], start=True, stop=True)
                                exps = ep.tile([128, 2, SCH], f16, name="exps", tag="e")
                                nc.scalar.activation(out=exps[:, :, cs_], in_=s_ps[:, :, cs_],
                                                     func=EXP, scale=0.125)
                                if m >= 0:
                                    nc.vector.tensor_mul(exps[:, :, cs_], exps[:, :, cs_],
                                                         masks[m][:, :, cs_])
                                pend.append((exps, t, c0))
                                if t % 2 == 1:
                                    # PVs first (one sps->ops switch), then
                                    # fillers (128-row like ops: free).
                                    while len(pend) > 2:
                                        e_, t_, c_ = pend.pop(0)
                                        flush_early_for(t_)
                                        emit_pv(e_, t_, c_)
                                    if early_left is not None:
                                        for _ in range(2 * early_rate):
                                            next(early_left, None)
                                    if bg_p1 is not None:
                                        for _ in range(2):
                                            next(bg_p1, None)
                                    for _ in range(2 * nd):
                                        next(bg, None)
                            for e_, t_, c_ in pend:
                                flush_early_for(t_)
                                emit_pv(e_, t_, c_)

                            # normalize: att = O[0:64] * bcast(1/denom).
                            # Head-ordered chains so head 0's o_ps releases
                            # (and the next pair's PV unblocks, with op
                            # bufs=3) after ONE chain latency, not two.
                            for h in range(2):
                                bc = bp.tile([128, SCH], f32, name=f"bc{h}", tag="bc")
                                nc.vector.tensor_copy(out=bc[0:1, :], in_=o_ps[h][64:65, :])
                                nc.vector.reciprocal_approx_fast(
                                    out=bc[0:1, :], in_=bc[0:1, :])
                                nc.gpsimd.partition_broadcast(
                                    out_ap=bc[0:64, :], in_ap=bc[0:1, :])
                                nc.vector.tensor_mul(
                                    att_sb[j][pair][64 * h:64 * (h + 1), :],
                                    o_ps[h][0:64, :], bc[0:64, :])

                    def drain_all(bg):
                        for _ in bg:
                            pass

                    def chain(*gens):
                        for g in gens:
                            yield from g

                    # Only pair 0's Q/K half runs before attn(0); pair 1's
                    # half and V(0) drain inside attn(0) itself (V paced one
                    # group ahead of its PV). Later chunks drain V(j) early
                    # (their straddle tiles read it) plus outp(j-1) and
                    # QK(j+1) quanta between k-tiles — keeps PE fed in the
                    # late, filler-starved chunks and shrinks the serial
                    # prologue at each rep boundary.
                    drain_all(qk_gen(0, halves=(0,)))
                    for j in range(NSCH):
                        gens = []
                        if j > 0:
                            gens.append(outp_gen(j - 1))
                        if j + 1 < NSCH:
                            gens.append(qk_gen(j + 1))
                        bg = chain(*gens)
                        attn(j, bg, bg_early=v_gen(j),
                             early_rate=5 if j == 0 else 2,
                             bg_p1=qk_gen(0, halves=(1,)) if j == 0 else None)
                        drain_all(bg)
                        if dma == "pipe":
                            # reload regions whose last reader just drained:
                            # x[j] (v_gen(j)); after attn(2) also Wq/Wk+bias
                            # (qk_gen(3) drained inside attn(2)); after
                            # attn(3) also Wv (v_gen(3)) and Wo. Wo's reload
                            # sits BEFORE outp(3), which then reads the
                            # freshly-loaded (identical) bytes — no DMA may
                            # trail the body: the loop back-edge tracks
                            # big_sb coarsely, so a post-body write would
                            # stall the next rep's first matmuls (~8us).
                            reload(XT_O + 4096 * j, XT_O + 4096 * (j + 1))
                            if j == 2:
                                reload(WQ_O, WV_O)
                                nc.sync.dma_start(out=bqk_sb, in_=bqk)
                            if j == 3:
                                reload(WV_O, WO_O)
                    drain_all(outp_gen(NSCH - 1))

            if reps == 1:
                body(0)
            elif reps % 4 == 0:
                # x4 unroll: the ~7us For_i back-edge ceremony is paid once
                # per FOUR reps (in-stream body seams are only ~780ns).
                with tc.For_i(0, reps // 4, 1):
                    body(0)
                    body(1)
                    body(0)
                    body(1)
            elif reps % 2 == 0:
                # body unrolled x2 per hardware-loop iteration: the For_i
                # back-edge costs ~7us (per-engine drains/branch + waiting
                # the last DMA's completion for sem-lane recycling) — paid
                # once per TWO reps this way. The body-to-body seam inside
                # an iteration is a plain instruction-stream transition the
                # Tile scheduler can overlap across. Measured 143.4 ->
                # 141.5us/rep.
                with tc.For_i(0, reps // 2, 1):
                    body(0)
                    body(1)
            else:
                with tc.For_i(0, reps, 1):
                    body()

    nc.compile()
    return nc


def _get_nc(reps=1, **kw):
    key = (reps, tuple(sorted(kw.items())))
    if key not in _cache:
        _cache[key] = _build(reps, **kw)
    return _cache[key]


def _tiles(a, nt):
    # [nt*128, w] -> [128, nt*w] with [p, t*w:t*w+w] = a[128t+p, :]
    w = a.shape[1]
    return a.reshape(nt, 128, w).transpose(1, 0, 2).reshape(128, nt * w)


def make_in_maps(x, Wq, bq, Wk, bk, Wv, bv, Wo, bo):
    """Shard full inputs into 8 per-core input dicts (fp16 payload)."""
    in_maps = []
    for core in range(N_CORES):
        b, g = core // 4, core % 4
        sl = slice(DSL * g, DSL * (g + 1))
        # x s-major: cols [sc, c, s] with xt[p, sc, c, s] = x[b].T[128c+p, 512sc+s]
        xsm = x[b].T.reshape(8, 128, 4, 512).transpose(1, 2, 0, 3).reshape(128, 16384)
        big = np.concatenate([
            xsm,
            _tiles(np.ascontiguousarray(Wq[sl, :].T), 8),
            _tiles(np.ascontiguousarray(Wk[sl, :].T), 8),
            _tiles(np.ascontiguousarray(Wv[sl, :].T), 8),
            _tiles(np.ascontiguousarray(Wo[:, sl].T), 2),
        ], axis=1).astype(np.float16)
        bqk = np.concatenate([bq[sl].reshape(2, 128).T, bk[sl].reshape(2, 128).T],
                             axis=1)
        in_maps.append({"big": big, "bqk": np.ascontiguousarray(bqk)})
    return in_maps


def kernel(x, Wq, bq, Wk, bk, Wv, bv, Wo, bo):
    from concourse.bass_utils import run_bass_kernel_spmd

    x = np.asarray(x, dtype=np.float32)
    Wq, bq = np.asarray(Wq, np.float32), np.asarray(bq, np.float32)
    Wk, bk = np.asarray(Wk, np.float32), np.asarray(bk, np.float32)
    Wv, bv = np.asarray(Wv, np.float32), np.asarray(bv, np.float32)
    Wo, bo = np.asarray(Wo, np.float32), np.asarray(bo, np.float32)

    nc = _get_nc()
    in_maps = make_in_maps(x, Wq, bq, Wk, bk, Wv, bv, Wo, bo)
    res = run_bass_kernel_spmd(nc, in_maps, core_ids=list(range(N_CORES)))

    cvec = bv @ Wo.T + bo  # x-independent bias contribution
    out = np.zeros((B, S, D), dtype=np.float32)
    for core in range(N_CORES):
        out[core // 4] += res.results[core]["y"].astype(np.float32)
    out += cvec[None, None, :]
    return out

